# revision 6
# baseline (speedup 1.0000x reference)
"""Distributed exact 1-NN (argmin L2) over 131072 vectors for 2048 queries.

Strategy (8 NeuronCores, vectors sharded along N):
  Host prep:
    - vsq = ||v||^2 globally sorted ascending; shard s takes sorted slice
      [s*16384, (s+1)*16384). Within each 2048-column device group the
      sorted vectors are interleaved so that the device's strided
      32-element max-blocks (positions n === j mod 64) correspond to 32
      CONSECUTIVE sorted vectors -> each block is vsq-homogeneous and
      blockmax(q.v) - min_vsq(block)/2 tightly bounds the block's best
      score (score = q.v - vsq/2; argmin dist == argmax score).
    - Per-core inputs: QT = query.T (bf16), VT = shard.T (bf16).
  Device (per core, SPMD):
    - S = QT.T @ VT in bf16 at full PE rate into PSUM [128, 2048] waves.
    - Per group, per-block max via either DVE strided tensor_reduce from
      PSUM, or ACT copy to SBUF bf16 + DVE tensor_tensor max fold tree
      (engine load balancing); PE warmup matmuls hide the HAM ramp.
    - Output blockmax [2048, 512] fp32 per core.
  Host post:
    - ub[block] = blockmax - 0.5*min_vsq(block); top-16 blocks/query over
      all 4096 blocks; exact fp32 rescore of their 512 vectors; fp64
      refine of the top 16; lowest-original-index tie-break -> int32.
"""

import sys

for _p in ('/opt/trn_rl_repo', '/opt/pypackages'):
    if _p not in sys.path:
        sys.path.append(_p)

import numpy as np

M, N, D = 2048, 131072, 256
N_CORES = 8
NSH = N // N_CORES          # 16384 vectors per core
BLK = 32                    # vectors per max-block
MT = 128                    # queries per m-tile
N_MT = M // MT              # 16 m-tiles
CHUNK = 512                 # matmul free dim (1 PSUM bank)
GROUP = 2048                # cols per PSUM wave (4 banks)
N_GROUP = NSH // GROUP      # 8 groups
NB = GROUP // BLK           # 64 blocks per group
NBLK = NSH // BLK           # 512 blocks per core
TOPB = 16                   # blocks rescored per query
DVE_DIRECT_GROUPS = {4}     # rest: ACT copy + DVE fold tree
N_WARMUP_MM = 16            # junk matmuls to ride out HAM ramp + input DMA

LAST_EXEC_NS = None
_CACHE = {}

# position n inside a group holds sorted-vector i(n) = (n%64)*32 + n//64,
# so strided set {n : n%64 == j} == sorted run [32j, 32j+32).
_n_idx = np.arange(GROUP)
_GROUP_PERM = (_n_idx % NB) * BLK + _n_idx // NB   # position -> sorted idx


def _build():
    import concourse.bacc as bacc
    import concourse.mybir as mybir
    from concourse import tile

    nc = bacc.Bacc("TRN2", target_bir_lowering=False, debug=False)
    f32 = mybir.dt.float32
    bf16 = mybir.dt.bfloat16

    qt_d = nc.dram_tensor("qt", [D, M], bf16, kind="ExternalInput")
    vt_d = nc.dram_tensor("vt", [D, NSH], bf16, kind="ExternalInput")
    bm_d = nc.dram_tensor("bm", [M, NBLK], bf16, kind="ExternalOutput")

    with tile.TileContext(nc) as tc:
        with tc.tile_pool(name="qtp", bufs=1) as qtp, \
             tc.tile_pool(name="vtp", bufs=1) as vtp, \
             tc.tile_pool(name="bmp", bufs=3) as bmp, \
             tc.tile_pool(name="cpp", bufs=6) as cpp, \
             tc.tile_pool(name="jkp", bufs=1) as jkp, \
             tc.tile_pool(name="psp", bufs=2, space="PSUM") as psp:

            # PE warmup on junk data while inputs DMA in (HAM ramp ~3.4us)
            junk = jkp.tile([128, 128 + CHUNK], bf16, tag="junk")
            nc.vector.memset(junk[:], 0.0)
            wps = psp.tile([128, GROUP], f32, tag="ps")
            for _ in range(N_WARMUP_MM):
                nc.tensor.matmul(wps[:, :CHUNK], junk[:, :128], junk[:, 128:],
                                 start=True, stop=True)

            qt_t = qtp.tile([128, 2, M], bf16, tag="qt")
            nc.sync.dma_start(
                qt_t[:], qt_d.ap().rearrange("(a p) m -> p a m", p=128))
            vt_tiles = []
            for g in range(N_GROUP):
                vt_t = vtp.tile([128, 2, GROUP], bf16, tag=f"vt{g}")
                nc.sync.dma_start(
                    vt_t[:],
                    vt_d.ap()[:, g * GROUP:(g + 1) * GROUP]
                    .rearrange("(a p) n -> p a n", p=128))
                vt_tiles.append(vt_t)

            for mt in range(N_MT):
                bm_t = bmp.tile([128, NBLK], bf16, tag="bm")
                for g in range(N_GROUP):
                    ps = psp.tile([128, GROUP], f32, tag="ps")
                    for c in range(GROUP // CHUNK):
                        for k in range(2):
                            nc.tensor.matmul(
                                ps[:, c * CHUNK:(c + 1) * CHUNK],
                                qt_t[:, k, mt * MT:(mt + 1) * MT],
                                vt_tiles[g][:, k, c * CHUNK:(c + 1) * CHUNK],
                                start=(k == 0), stop=(k == 1))
                    bslice = bm_t[:, g * NB:(g + 1) * NB]
                    if g in DVE_DIRECT_GROUPS:
                        # strided blocks: out[j] = max_w ps[w*64 + j]
                        nc.vector.tensor_reduce(
                            out=bslice,
                            in_=ps[:].rearrange("p (w b) -> p b w", b=NB),
                            op=mybir.AluOpType.max, axis=mybir.AxisListType.X)
                    else:
                        cp = cpp.tile([128, GROUP], bf16, tag="cp")
                        nc.scalar.copy(cp[:], ps[:])
                        w = GROUP // 2
                        while w >= NB * 2:
                            nc.vector.tensor_tensor(
                                out=cp[:, :w], in0=cp[:, :w], in1=cp[:, w:2 * w],
                                op=mybir.AluOpType.max)
                            w //= 2
                        nc.vector.tensor_tensor(
                            out=bslice, in0=cp[:, :NB], in1=cp[:, NB:2 * NB],
                            op=mybir.AluOpType.max)
                nc.sync.dma_start(bm_d.ap()[mt * MT:(mt + 1) * MT, :], bm_t[:])
    nc.compile()
    return nc


def _get_nc():
    if "nc" not in _CACHE:
        _CACHE["nc"] = _build()
    return _CACHE["nc"]


def kernel(query: np.ndarray, vectors: np.ndarray, _trace: bool = False) -> np.ndarray:
    global LAST_EXEC_NS
    from concourse.bass_utils import run_bass_kernel_spmd
    import ml_dtypes

    query = np.ascontiguousarray(query, dtype=np.float32)
    vectors = np.ascontiguousarray(vectors, dtype=np.float32)

    # --- host prep ---------------------------------------------------
    vsq64 = (vectors.astype(np.float64) ** 2).sum(axis=1)
    order = np.argsort(vsq64, kind="stable").astype(np.int64)
    # interleave within each 2048-group so device strided blocks are
    # consecutive sorted runs
    layout = order.reshape(-1, GROUP)[:, _GROUP_PERM].reshape(-1)  # [N]
    lv = vectors[layout]                       # device column order
    qt = np.ascontiguousarray(query.T).astype(ml_dtypes.bfloat16)

    in_maps = []
    for s in range(N_CORES):
        sh = lv[s * NSH:(s + 1) * NSH]
        in_maps.append({
            "qt": np.asarray(qt),
            "vt": np.asarray(np.ascontiguousarray(sh.T).astype(ml_dtypes.bfloat16)),
        })

    # per-block (32 consecutive sorted) min vsq for ub correction
    svsq = vsq64[order]
    blk_min_vsq = svsq.reshape(N_CORES * NBLK, BLK).min(axis=1)   # [4096]

    # --- device ------------------------------------------------------
    nc = _get_nc()
    res = run_bass_kernel_spmd(
        nc, in_maps, core_ids=list(range(N_CORES)), trace=_trace)
    if _trace:
        LAST_EXEC_NS = res.exec_time_ns
    # bm[core][m, g*64+j] = blockmax of sorted block (core, g, j)
    bm = np.concatenate(
        [np.asarray(res.results[s]["bm"]).astype(np.float32)[:, None, :]
         for s in range(N_CORES)], axis=1
    ).reshape(M, N_CORES * NBLK)               # [M, 4096]

    # --- host post: rank blocks, exact rescore -----------------------
    ub = bm - 0.5 * blk_min_vsq[None, :].astype(np.float32)
    top_blocks = np.argpartition(-ub, TOPB, axis=1)[:, :TOPB]      # [M, TOPB]

    sorted_ids = order.reshape(N_CORES * NBLK, BLK)                # block -> orig ids
    q64 = query.astype(np.float64)
    sv = vectors[order]                        # sorted-order vectors (f32)

    out = np.empty(M, dtype=np.int64)
    CH = 256
    srows = np.arange(CH)[:, None]
    for i in range(0, M, CH):
        tb = top_blocks[i:i + CH]                                  # [ch, TOPB]
        # candidate sorted positions: block b covers sorted [b*32, b*32+32)
        cand = (tb[:, :, None] * BLK + np.arange(BLK)[None, None, :]
                ).reshape(-1, TOPB * BLK)                          # [ch, 512]
        cv = sv[cand]                                              # [ch, 512, D]
        d32 = ((cv - query[i:i + CH, None, :]) ** 2).sum(axis=2)
        part = np.argpartition(d32, 16, axis=1)[:, :16]
        cv64 = cv[srows, part].astype(np.float64)
        d64 = ((cv64 - q64[i:i + CH, None, :]) ** 2).sum(axis=2)
        gids = order[cand[srows, part]]                            # original ids
        best = np.lexsort((gids, d64), axis=1)[:, 0]
        out[i:i + CH] = gids[srows[:, 0], best]

    return out.astype(np.int32)


# revision 7
# speedup vs baseline: 1.0029x; 1.0029x over previous
"""Distributed exact 1-NN (argmin L2) over 131072 vectors for 2048 queries.

Strategy (8 NeuronCores, vectors sharded along N):
  Host prep:
    - vsq = ||v||^2 globally sorted ascending; shard s takes sorted slice
      [s*16384, (s+1)*16384). Within each 2048-column device group the
      sorted vectors are interleaved so that the device's strided
      32-element max-blocks (positions n === j mod 64) correspond to 32
      CONSECUTIVE sorted vectors -> each block is vsq-homogeneous and
      blockmax(q.v) - min_vsq(block)/2 tightly bounds the block's best
      score (score = q.v - vsq/2; argmin dist == argmax score).
    - Per-core inputs: QT = query.T (bf16), VT = shard.T (bf16).
  Device (per core, SPMD):
    - S = QT.T @ VT in bf16 at full PE rate into PSUM [128, 2048] waves.
    - Per group, per-block max via either DVE strided tensor_reduce from
      PSUM, or ACT copy to SBUF bf16 + DVE tensor_tensor max fold tree
      (engine load balancing); PE warmup matmuls hide the HAM ramp.
    - Output blockmax [2048, 512] fp32 per core.
  Host post:
    - ub[block] = blockmax - 0.5*min_vsq(block); top-16 blocks/query over
      all 4096 blocks; exact fp32 rescore of their 512 vectors; fp64
      refine of the top 16; lowest-original-index tie-break -> int32.
"""

import sys

for _p in ('/opt/trn_rl_repo', '/opt/pypackages'):
    if _p not in sys.path:
        sys.path.append(_p)

import numpy as np

M, N, D = 2048, 131072, 256
N_CORES = 8
NSH = N // N_CORES          # 16384 vectors per core
BLK = 32                    # vectors per max-block
MT = 128                    # queries per m-tile
N_MT = M // MT              # 16 m-tiles
CHUNK = 512                 # matmul free dim (1 PSUM bank)
GROUP = 2048                # cols per PSUM wave (4 banks)
N_GROUP = NSH // GROUP      # 8 groups
NB = GROUP // BLK           # 64 blocks per group
NBLK = NSH // BLK           # 512 blocks per core
TOPB = 16                   # blocks rescored per query
DVE_DIRECT_GROUPS = {4}     # rest: ACT copy + DVE fold tree
N_WARMUP_MM = 16            # junk matmuls to ride out HAM ramp + input DMA

LAST_EXEC_NS = None
_CACHE = {}

# position n inside a group holds sorted-vector i(n) = (n%64)*32 + n//64,
# so strided set {n : n%64 == j} == sorted run [32j, 32j+32).
_n_idx = np.arange(GROUP)
_GROUP_PERM = (_n_idx % NB) * BLK + _n_idx // NB   # position -> sorted idx


def _build():
    import concourse.bacc as bacc
    import concourse.mybir as mybir
    from concourse import tile

    nc = bacc.Bacc("TRN2", target_bir_lowering=False, debug=False)
    f32 = mybir.dt.float32
    bf16 = mybir.dt.bfloat16

    qt_d = nc.dram_tensor("qt", [D, M], bf16, kind="ExternalInput")
    vt_d = nc.dram_tensor("vt", [D, NSH], bf16, kind="ExternalInput")
    bm_d = nc.dram_tensor("bm", [M, NBLK], bf16, kind="ExternalOutput")

    with tile.TileContext(nc) as tc:
        with tc.tile_pool(name="qtp", bufs=1) as qtp, \
             tc.tile_pool(name="vtp", bufs=1) as vtp, \
             tc.tile_pool(name="bmp", bufs=3) as bmp, \
             tc.tile_pool(name="cpp", bufs=8) as cpp, \
             tc.tile_pool(name="jkp", bufs=1) as jkp, \
             tc.tile_pool(name="psp", bufs=2, space="PSUM") as psp:

            # PE warmup on junk data while inputs DMA in (HAM ramp ~3.4us)
            junk = jkp.tile([128, 128 + CHUNK], bf16, tag="junk")
            nc.vector.memset(junk[:], 0.0)
            wps = psp.tile([128, GROUP], f32, tag="ps")
            for _ in range(N_WARMUP_MM):
                nc.tensor.matmul(wps[:, :CHUNK], junk[:, :128], junk[:, 128:],
                                 start=True, stop=True)

            qt_t = qtp.tile([128, 2, M], bf16, tag="qt")
            nc.sync.dma_start(
                qt_t[:], qt_d.ap().rearrange("(a p) m -> p a m", p=128))
            vt_tiles = []
            for g in range(N_GROUP):
                vt_t = vtp.tile([128, 2, GROUP], bf16, tag=f"vt{g}")
                nc.sync.dma_start(
                    vt_t[:],
                    vt_d.ap()[:, g * GROUP:(g + 1) * GROUP]
                    .rearrange("(a p) n -> p a n", p=128))
                vt_tiles.append(vt_t)

            for mt in range(N_MT):
                bm_t = bmp.tile([128, NBLK], bf16, tag="bm")
                for g in range(N_GROUP):
                    ps = psp.tile([128, GROUP], f32, tag="ps")
                    for c in range(GROUP // CHUNK):
                        for k in range(2):
                            nc.tensor.matmul(
                                ps[:, c * CHUNK:(c + 1) * CHUNK],
                                qt_t[:, k, mt * MT:(mt + 1) * MT],
                                vt_tiles[g][:, k, c * CHUNK:(c + 1) * CHUNK],
                                start=(k == 0), stop=(k == 1))
                    bslice = bm_t[:, g * NB:(g + 1) * NB]
                    if g in DVE_DIRECT_GROUPS or (mt == N_MT - 1 and g >= 6):
                        # strided blocks: out[j] = max_w ps[w*64 + j]
                        nc.vector.tensor_reduce(
                            out=bslice,
                            in_=ps[:].rearrange("p (w b) -> p b w", b=NB),
                            op=mybir.AluOpType.max, axis=mybir.AxisListType.X)
                    else:
                        cp = cpp.tile([128, GROUP], bf16, tag="cp")
                        nc.scalar.copy(cp[:], ps[:])
                        w = GROUP // 2
                        while w >= NB * 2:
                            nc.vector.tensor_tensor(
                                out=cp[:, :w], in0=cp[:, :w], in1=cp[:, w:2 * w],
                                op=mybir.AluOpType.max)
                            w //= 2
                        nc.vector.tensor_tensor(
                            out=bslice, in0=cp[:, :NB], in1=cp[:, NB:2 * NB],
                            op=mybir.AluOpType.max)
                nc.sync.dma_start(bm_d.ap()[mt * MT:(mt + 1) * MT, :], bm_t[:])
    nc.compile()
    return nc


def _get_nc():
    if "nc" not in _CACHE:
        _CACHE["nc"] = _build()
    return _CACHE["nc"]


def kernel(query: np.ndarray, vectors: np.ndarray, _trace: bool = False) -> np.ndarray:
    global LAST_EXEC_NS
    from concourse.bass_utils import run_bass_kernel_spmd
    import ml_dtypes

    query = np.ascontiguousarray(query, dtype=np.float32)
    vectors = np.ascontiguousarray(vectors, dtype=np.float32)

    # --- host prep ---------------------------------------------------
    vsq64 = (vectors.astype(np.float64) ** 2).sum(axis=1)
    order = np.argsort(vsq64, kind="stable").astype(np.int64)
    # interleave within each 2048-group so device strided blocks are
    # consecutive sorted runs
    layout = order.reshape(-1, GROUP)[:, _GROUP_PERM].reshape(-1)  # [N]
    lv = vectors[layout]                       # device column order
    qt = np.ascontiguousarray(query.T).astype(ml_dtypes.bfloat16)

    in_maps = []
    for s in range(N_CORES):
        sh = lv[s * NSH:(s + 1) * NSH]
        in_maps.append({
            "qt": np.asarray(qt),
            "vt": np.asarray(np.ascontiguousarray(sh.T).astype(ml_dtypes.bfloat16)),
        })

    # per-block (32 consecutive sorted) min vsq for ub correction
    svsq = vsq64[order]
    blk_min_vsq = svsq.reshape(N_CORES * NBLK, BLK).min(axis=1)   # [4096]

    # --- device ------------------------------------------------------
    nc = _get_nc()
    res = run_bass_kernel_spmd(
        nc, in_maps, core_ids=list(range(N_CORES)), trace=_trace)
    if _trace:
        LAST_EXEC_NS = res.exec_time_ns
    # bm[core][m, g*64+j] = blockmax of sorted block (core, g, j)
    bm = np.concatenate(
        [np.asarray(res.results[s]["bm"]).astype(np.float32)[:, None, :]
         for s in range(N_CORES)], axis=1
    ).reshape(M, N_CORES * NBLK)               # [M, 4096]

    # --- host post: rank blocks, exact rescore -----------------------
    ub = bm - 0.5 * blk_min_vsq[None, :].astype(np.float32)
    top_blocks = np.argpartition(-ub, TOPB, axis=1)[:, :TOPB]      # [M, TOPB]

    sorted_ids = order.reshape(N_CORES * NBLK, BLK)                # block -> orig ids
    q64 = query.astype(np.float64)
    sv = vectors[order]                        # sorted-order vectors (f32)

    out = np.empty(M, dtype=np.int64)
    CH = 256
    srows = np.arange(CH)[:, None]
    for i in range(0, M, CH):
        tb = top_blocks[i:i + CH]                                  # [ch, TOPB]
        # candidate sorted positions: block b covers sorted [b*32, b*32+32)
        cand = (tb[:, :, None] * BLK + np.arange(BLK)[None, None, :]
                ).reshape(-1, TOPB * BLK)                          # [ch, 512]
        cv = sv[cand]                                              # [ch, 512, D]
        d32 = ((cv - query[i:i + CH, None, :]) ** 2).sum(axis=2)
        part = np.argpartition(d32, 16, axis=1)[:, :16]
        cv64 = cv[srows, part].astype(np.float64)
        d64 = ((cv64 - q64[i:i + CH, None, :]) ** 2).sum(axis=2)
        gids = order[cand[srows, part]]                            # original ids
        best = np.lexsort((gids, d64), axis=1)[:, 0]
        out[i:i + CH] = gids[srows[:, 0], best]

    return out.astype(np.int32)


# revision 8
# speedup vs baseline: 1.0183x; 1.0154x over previous
"""Distributed exact 1-NN (argmin L2) over 131072 vectors for 2048 queries.

Strategy (8 NeuronCores, vectors sharded along N):
  Host prep:
    - vsq = ||v||^2 globally sorted ascending; shard s takes sorted slice
      [s*16384, (s+1)*16384). Within each 2048-column device group the
      sorted vectors are interleaved so that the device's strided
      32-element max-blocks (positions n === j mod 64) correspond to 32
      CONSECUTIVE sorted vectors -> each block is vsq-homogeneous and
      blockmax(q.v) - min_vsq(block)/2 tightly bounds the block's best
      score (score = q.v - vsq/2; argmin dist == argmax score).
    - Per-core inputs: QT = query.T (bf16), VT = shard.T (bf16).
  Device (per core, SPMD):
    - S = QT.T @ VT in bf16 at full PE rate into PSUM [128, 2048] waves.
    - Per group, per-block max via either DVE strided tensor_reduce from
      PSUM, or ACT copy to SBUF bf16 + DVE tensor_tensor max fold tree
      (engine load balancing); PE warmup matmuls hide the HAM ramp.
    - Output blockmax [2048, 512] fp32 per core.
  Host post:
    - ub[block] = blockmax - 0.5*min_vsq(block); top-16 blocks/query over
      all 4096 blocks; exact fp32 rescore of their 512 vectors; fp64
      refine of the top 16; lowest-original-index tie-break -> int32.
"""

import sys

for _p in ('/opt/trn_rl_repo', '/opt/pypackages'):
    if _p not in sys.path:
        sys.path.append(_p)

import numpy as np

M, N, D = 2048, 131072, 256
N_CORES = 8
NSH = N // N_CORES          # 16384 vectors per core
BLK = 32                    # vectors per max-block
MT = 128                    # queries per m-tile
N_MT = M // MT              # 16 m-tiles
CHUNK = 512                 # matmul free dim (1 PSUM bank)
GROUP = 2048                # cols per PSUM wave (4 banks)
N_GROUP = NSH // GROUP      # 8 groups
NB = GROUP // BLK           # 64 blocks per group
NBLK = NSH // BLK           # 512 blocks per core
TOPB = 16                   # blocks rescored per query
DVE_DIRECT_GROUPS = {4}     # rest: ACT copy + DVE fold tree
N_WARMUP_MM = 24            # junk matmuls to ride out HAM ramp + input DMA

LAST_EXEC_NS = None
_CACHE = {}

# position n inside a group holds sorted-vector i(n) = (n%64)*32 + n//64,
# so strided set {n : n%64 == j} == sorted run [32j, 32j+32).
_n_idx = np.arange(GROUP)
_GROUP_PERM = (_n_idx % NB) * BLK + _n_idx // NB   # position -> sorted idx


def _build():
    import concourse.bacc as bacc
    import concourse.mybir as mybir
    from concourse import tile

    nc = bacc.Bacc("TRN2", target_bir_lowering=False, debug=False)
    f32 = mybir.dt.float32
    bf16 = mybir.dt.bfloat16

    qt_d = nc.dram_tensor("qt", [D, M], bf16, kind="ExternalInput")
    vt_d = nc.dram_tensor("vt", [D, NSH], bf16, kind="ExternalInput")
    bm_d = nc.dram_tensor("bm", [M, NBLK], bf16, kind="ExternalOutput")

    with tile.TileContext(nc) as tc:
        with tc.tile_pool(name="qtp", bufs=1) as qtp, \
             tc.tile_pool(name="vtp", bufs=1) as vtp, \
             tc.tile_pool(name="bmp", bufs=3) as bmp, \
             tc.tile_pool(name="cpp", bufs=8) as cpp, \
             tc.tile_pool(name="jkp", bufs=1) as jkp, \
             tc.tile_pool(name="psp", bufs=2, space="PSUM") as psp:

            # PE warmup on junk data while inputs DMA in (HAM ramp ~3.4us)
            junk = jkp.tile([128, 128 + CHUNK], bf16, tag="junk")
            nc.vector.memset(junk[:], 0.0)
            wps = psp.tile([128, GROUP], f32, tag="ps")
            for _ in range(N_WARMUP_MM):
                nc.tensor.matmul(wps[:, :CHUNK], junk[:, :128], junk[:, 128:],
                                 start=True, stop=True)

            qt_t = qtp.tile([128, 2, M], bf16, tag="qt")
            nc.sync.dma_start(
                qt_t[:], qt_d.ap().rearrange("(a p) m -> p a m", p=128))
            vt_tiles = []
            for g in range(N_GROUP):
                vt_t = vtp.tile([128, 2, GROUP], bf16, tag=f"vt{g}")
                nc.sync.dma_start(
                    vt_t[:],
                    vt_d.ap()[:, g * GROUP:(g + 1) * GROUP]
                    .rearrange("(a p) n -> p a n", p=128))
                vt_tiles.append(vt_t)

            for mt in range(N_MT):
                bm_t = bmp.tile([128, NBLK], bf16, tag="bm")
                for g in range(N_GROUP):
                    ps = psp.tile([128, GROUP], f32, tag="ps")
                    for c in range(GROUP // CHUNK):
                        for k in range(2):
                            nc.tensor.matmul(
                                ps[:, c * CHUNK:(c + 1) * CHUNK],
                                qt_t[:, k, mt * MT:(mt + 1) * MT],
                                vt_tiles[g][:, k, c * CHUNK:(c + 1) * CHUNK],
                                start=(k == 0), stop=(k == 1))
                    bslice = bm_t[:, g * NB:(g + 1) * NB]
                    if g in DVE_DIRECT_GROUPS or (mt == N_MT - 1 and g >= 6):
                        # strided blocks: out[j] = max_w ps[w*64 + j]
                        nc.vector.tensor_reduce(
                            out=bslice,
                            in_=ps[:].rearrange("p (w b) -> p b w", b=NB),
                            op=mybir.AluOpType.max, axis=mybir.AxisListType.X)
                    else:
                        cp = cpp.tile([128, GROUP], bf16, tag="cp")
                        nc.scalar.copy(cp[:], ps[:])
                        w = GROUP // 2
                        while w >= NB * 2:
                            nc.vector.tensor_tensor(
                                out=cp[:, :w], in0=cp[:, :w], in1=cp[:, w:2 * w],
                                op=mybir.AluOpType.max)
                            w //= 2
                        nc.vector.tensor_tensor(
                            out=bslice, in0=cp[:, :NB], in1=cp[:, NB:2 * NB],
                            op=mybir.AluOpType.max)
                nc.sync.dma_start(bm_d.ap()[mt * MT:(mt + 1) * MT, :], bm_t[:])
    nc.compile()
    return nc


def _get_nc():
    if "nc" not in _CACHE:
        _CACHE["nc"] = _build()
    return _CACHE["nc"]


def kernel(query: np.ndarray, vectors: np.ndarray, _trace: bool = False) -> np.ndarray:
    global LAST_EXEC_NS
    from concourse.bass_utils import run_bass_kernel_spmd
    import ml_dtypes

    query = np.ascontiguousarray(query, dtype=np.float32)
    vectors = np.ascontiguousarray(vectors, dtype=np.float32)

    # --- host prep ---------------------------------------------------
    vsq64 = (vectors.astype(np.float64) ** 2).sum(axis=1)
    order = np.argsort(vsq64, kind="stable").astype(np.int64)
    # interleave within each 2048-group so device strided blocks are
    # consecutive sorted runs
    layout = order.reshape(-1, GROUP)[:, _GROUP_PERM].reshape(-1)  # [N]
    lv = vectors[layout]                       # device column order
    qt = np.ascontiguousarray(query.T).astype(ml_dtypes.bfloat16)

    in_maps = []
    for s in range(N_CORES):
        sh = lv[s * NSH:(s + 1) * NSH]
        in_maps.append({
            "qt": np.asarray(qt),
            "vt": np.asarray(np.ascontiguousarray(sh.T).astype(ml_dtypes.bfloat16)),
        })

    # per-block (32 consecutive sorted) min vsq for ub correction
    svsq = vsq64[order]
    blk_min_vsq = svsq.reshape(N_CORES * NBLK, BLK).min(axis=1)   # [4096]

    # --- device ------------------------------------------------------
    nc = _get_nc()
    res = run_bass_kernel_spmd(
        nc, in_maps, core_ids=list(range(N_CORES)), trace=_trace)
    if _trace:
        LAST_EXEC_NS = res.exec_time_ns
    # bm[core][m, g*64+j] = blockmax of sorted block (core, g, j)
    bm = np.concatenate(
        [np.asarray(res.results[s]["bm"]).astype(np.float32)[:, None, :]
         for s in range(N_CORES)], axis=1
    ).reshape(M, N_CORES * NBLK)               # [M, 4096]

    # --- host post: rank blocks, exact rescore -----------------------
    ub = bm - 0.5 * blk_min_vsq[None, :].astype(np.float32)
    top_blocks = np.argpartition(-ub, TOPB, axis=1)[:, :TOPB]      # [M, TOPB]

    sorted_ids = order.reshape(N_CORES * NBLK, BLK)                # block -> orig ids
    q64 = query.astype(np.float64)
    sv = vectors[order]                        # sorted-order vectors (f32)

    out = np.empty(M, dtype=np.int64)
    CH = 256
    srows = np.arange(CH)[:, None]
    for i in range(0, M, CH):
        tb = top_blocks[i:i + CH]                                  # [ch, TOPB]
        # candidate sorted positions: block b covers sorted [b*32, b*32+32)
        cand = (tb[:, :, None] * BLK + np.arange(BLK)[None, None, :]
                ).reshape(-1, TOPB * BLK)                          # [ch, 512]
        cv = sv[cand]                                              # [ch, 512, D]
        d32 = ((cv - query[i:i + CH, None, :]) ** 2).sum(axis=2)
        part = np.argpartition(d32, 16, axis=1)[:, :16]
        cv64 = cv[srows, part].astype(np.float64)
        d64 = ((cv64 - q64[i:i + CH, None, :]) ** 2).sum(axis=2)
        gids = order[cand[srows, part]]                            # original ids
        best = np.lexsort((gids, d64), axis=1)[:, 0]
        out[i:i + CH] = gids[srows[:, 0], best]

    return out.astype(np.int32)
